# revision 8
# baseline (speedup 1.0000x reference)
"""ARCAttention (MLA + pattern-attention + gate) distributed Bass kernel for 8 TRN2 NeuronCores.

Sharding: data-parallel over batch (B=2) x tensor-parallel over heads (4 head-groups).
Core c handles batch (c // 4), heads [4*(c%4) .. 4*(c%4)+4) of both the MLA path and the
pattern path. The low-rank a-projections (q_a, kv_a lora) and the gate are replicated
within a batch group. Each core emits a partial (already gate-weighted) output
[S, HID]; the host sums the 4 partials per batch. No device collectives.

All matmuls run in bf16 (f32 PSUM accumulation); softmax/rmsnorm statistics in f32.
Weight preprocessing (transposes, ln-weight folding, scale folding, rope tables) is
done on host in numpy and shipped per-core via in_maps.
"""

import numpy as np
import ml_dtypes

# ---- model config (hardcoded from the problem spec) ----
B, S, HID = 2, 1024, 2048
H = 16
D_NOPE, D_ROPE, D_V = 128, 64, 128
D_Q = D_NOPE + D_ROPE            # 192
QR, KVR = 1536, 512
PH, PD = 16, 128
THETA, EPS = 10000.0, 1e-6
NCORES = 8
HPC = 4                          # heads per core
TB = S // 128                    # 8 token blocks
KT_HID = HID // 128              # 16
KT_QR = QR // 128                # 12
KT_KVR = KVR // 128              # 4

BF16 = ml_dtypes.bfloat16

# knobs for test harness
TRACE = False
RUN_KWARGS = {}
LAST_RESULT = None

_graph_cache = {}


def _build_graph():
    import concourse.bass as bass
    import concourse.mybir as mybir
    import concourse.tile as tile
    from concourse import bacc
    from concourse.masks import make_identity

    BF = mybir.dt.bfloat16
    F32 = mybir.dt.float32
    Exp = mybir.ActivationFunctionType.Exp
    Square = mybir.ActivationFunctionType.Square
    Sqrt = mybir.ActivationFunctionType.Sqrt
    MULT = mybir.AluOpType.mult
    ADD = mybir.AluOpType.add
    X = mybir.AxisListType.X
    ts = bass.ts

    nc = bacc.Bacc("TRN2", target_bir_lowering=False, debug=False,
                   num_devices=NCORES)

    def din(name, shape, dt=BF):
        return nc.declare_dram_parameter(name, list(shape), dt, isOutput=False)

    xT_d = din("xT", [HID, S])
    qa_d = din("qa_wT", [HID, QR])
    qbn_d = din("qbn_wT", [QR, HPC * D_NOPE])
    qbp_d = din("qbp_wT", [QR, HPC * D_ROPE])
    kvl_d = din("kvl_wT", [HID, KVR])
    kvp_d = din("kvp_wT", [HID, HPC * D_ROPE])
    kbn_d = din("kbn_wT", [KVR, HPC * D_NOPE])
    kbv_d = din("kbv_wT", [KVR, HPC * D_V])
    ow_d = din("o_wT", [HPC * D_V, HID])
    spq_d = din("spq_wT", [HID, HPC * PD])
    spk_d = din("spk_wT", [HID, HPC * PD])
    spv_d = din("spv_wT", [HID, HPC * PD])
    spo_d = din("spo_wT", [HPC * PD, HID])
    gw_d = din("gate_wT", [HID, 2])
    gb_d = din("gate_bias", [128, 2], F32)
    cos_d = din("cos2T", [128, S])
    sin_d = din("sin2T", [128, S])
    out_d = nc.declare_dram_parameter("out", [S, HID], F32, isOutput=True)

    def r3(dram, kt):
        # [kt*128, N] dram tensor viewed as [128, kt, N] for SBUF tiling
        return dram.ap().rearrange("(k p) n -> p k n", p=128, k=kt)

    with tile.TileContext(nc) as tc:
        with (
            tc.tile_pool(name="const", bufs=1) as constp,
            tc.tile_pool(name="small", bufs=4) as small,
            tc.tile_pool(name="pp", bufs=4, space="PSUM") as pp,
            tc.tile_pool(name="pt", bufs=4, space="PSUM") as pt,
            tc.tile_pool(name="ain_mla", bufs=1) as ain,
        ):
            ident = constp.tile([128, 128], BF, tag="ident")
            make_identity(nc, ident[:])
            eps_t = constp.tile([128, 1], F32, tag="eps")
            nc.vector.memset(eps_t[:], EPS)
            cosT = constp.tile([128, S], BF, tag="cos")
            sinT = constp.tile([128, S], BF, tag="sin")
            nc.sync.dma_start(out=cosT[:], in_=cos_d.ap())
            nc.sync.dma_start(out=sinT[:], in_=sin_d.ap())
            gbias = constp.tile([128, 2], F32, tag="gb")
            nc.sync.dma_start(out=gbias[:], in_=gb_d.ap())
            g0_s = constp.tile([128, TB], F32, tag="g0")
            g1_s = constp.tile([128, TB], F32, tag="g1")
            ssq_q = constp.tile([128, TB * 3], F32, tag="ssq_q")
            ssq_k = constp.tile([128, TB], F32, tag="ssq_k")

            # MLA attention inputs (feature-major unless noted)
            qnopeT = ain.tile([128, HPC, S], BF, tag="qnopeT")
            qpeT = ain.tile([128, 2, S], BF, tag="qpeT")
            knopeT = ain.tile([128, HPC, S], BF, tag="knopeT")
            kpeT = ain.tile([128, 2, S], BF, tag="kpeT")
            v_s = ain.tile([128, TB, HPC * D_V], BF, tag="v")      # token-major

            def rope_from_psum(ps, dst, nck, work):
                """Apply rope to a [128, 512] psum chunk holding 2 stacked
                64-dim pe heads; write bf16 to dst ([128,512] slice)."""
                rot = work.tile([128, 512], F32, tag="rot")
                nc.vector.tensor_scalar_mul(rot[0:32, :], ps[32:64, :], -1.0)
                nc.vector.tensor_copy(rot[32:64, :], ps[0:32, :])
                nc.vector.tensor_scalar_mul(rot[64:96, :], ps[96:128, :], -1.0)
                nc.vector.tensor_copy(rot[96:128, :], ps[64:96, :])
                t1 = work.tile([128, 512], F32, tag="t1")
                nc.vector.tensor_mul(t1[:], ps[:], cosT[:, ts(nck, 512)])
                nc.vector.tensor_mul(rot[:], rot[:], sinT[:, ts(nck, 512)])
                nc.vector.tensor_add(dst, t1[:], rot[:])

            def softmax_rowstats(s0, s1):
                """Row max/sum prep over two [128,512] psum halves.
                Returns (nm, probs_writer) where nm is negated row max."""
                m0 = small.tile([128, 1], F32, tag="m0")
                m1 = small.tile([128, 1], F32, tag="m1")
                nc.vector.reduce_max(m0[:], s0[:], axis=X)
                nc.vector.reduce_max(m1[:], s1[:], axis=X)
                nm = small.tile([128, 1], F32, tag="nm")
                nc.vector.tensor_max(nm[:], m0[:], m1[:])
                nc.vector.tensor_scalar_mul(nm[:], nm[:], -1.0)
                return nm

            with tc.tile_pool(name="xp", bufs=1) as xp:
                xT = xp.tile([128, KT_HID, S], BF, tag="xT")
                nc.sync.dma_start(out=xT[:], in_=r3(xT_d, KT_HID))

                with tc.tile_pool(name="wrope", bufs=2) as wrope:
                    # ---------- Stage 1+2 ----------
                    with tc.tile_pool(name="q2", bufs=1) as q2:
                        qmidT = q2.tile([128, KT_QR, S], BF, tag="qmidT")
                        kvnT = q2.tile([128, KT_KVR, S], BF, tag="kvnT")

                        with (
                            tc.tile_pool(name="q1", bufs=1) as q1,
                            tc.tile_pool(name="w1", bufs=1) as w1,
                        ):
                            qmid = q1.tile([128, TB, QR], BF, tag="qmid")
                            kvn = q1.tile([128, TB, KVR], BF, tag="kvn")

                            # q_a: token-major [tok, QR] in 3 chunks of 512
                            for ck in range(3):
                                wt = w1.tile([128, KT_HID, 512], BF, tag="w")
                                nc.sync.dma_start(
                                    out=wt[:], in_=r3(qa_d, KT_HID)[:, :, ts(ck, 512)])
                                for tb in range(TB):
                                    ps = pp.tile([128, 512], F32, tag="pp")
                                    for k in range(KT_HID):
                                        nc.tensor.matmul(
                                            ps[:], lhsT=xT[:, k, ts(tb, 128)],
                                            rhs=wt[:, k, :],
                                            start=(k == 0), stop=(k == KT_HID - 1))
                                    nc.any.tensor_copy(qmid[:, tb, ts(ck, 512)], ps[:])
                                    nc.scalar.activation(
                                        ps[:], ps[:], Square,
                                        accum_out=ssq_q[:, tb * 3 + ck: tb * 3 + ck + 1])

                            # kv_a lora part: token-major [tok, KVR]
                            wt_kl = w1.tile([128, KT_HID, 512], BF, tag="w")
                            nc.sync.dma_start(out=wt_kl[:], in_=r3(kvl_d, KT_HID))
                            for tb in range(TB):
                                ps = pp.tile([128, 512], F32, tag="pp")
                                for k in range(KT_HID):
                                    nc.tensor.matmul(
                                        ps[:], lhsT=xT[:, k, ts(tb, 128)],
                                        rhs=wt_kl[:, k, :],
                                        start=(k == 0), stop=(k == KT_HID - 1))
                                nc.any.tensor_copy(kvn[:, tb, :], ps[:])
                                nc.scalar.activation(ps[:], ps[:], Square,
                                                     accum_out=ssq_k[:, tb:tb + 1])

                            # rmsnorm scales (in-place), then transpose to feature-major
                            for tb in range(TB):
                                acc = small.tile([128, 1], F32, tag="acc")
                                nc.vector.tensor_add(acc[:], ssq_q[:, tb * 3:tb * 3 + 1],
                                                     ssq_q[:, tb * 3 + 1:tb * 3 + 2])
                                nc.vector.tensor_add(acc[:], acc[:],
                                                     ssq_q[:, tb * 3 + 2:tb * 3 + 3])
                                rms = small.tile([128, 1], F32, tag="rms")
                                nc.scalar.activation(rms[:], acc[:], Sqrt,
                                                     bias=eps_t[:], scale=1.0 / QR)
                                inv = small.tile([128, 1], F32, tag="inv")
                                nc.vector.reciprocal(inv[:], rms[:])
                                nc.vector.tensor_scalar_mul(qmid[:, tb, :],
                                                            qmid[:, tb, :], inv[:])

                                rms2 = small.tile([128, 1], F32, tag="rms")
                                nc.scalar.activation(rms2[:], ssq_k[:, tb:tb + 1], Sqrt,
                                                     bias=eps_t[:], scale=1.0 / KVR)
                                inv2 = small.tile([128, 1], F32, tag="inv")
                                nc.vector.reciprocal(inv2[:], rms2[:])
                                nc.vector.tensor_scalar_mul(kvn[:, tb, :],
                                                            kvn[:, tb, :], inv2[:])

                                for kt in range(KT_QR):
                                    tp = pt.tile([128, 128], BF, tag="pt")
                                    nc.tensor.transpose(tp[:], qmid[:, tb, ts(kt, 128)],
                                                        ident[:])
                                    nc.any.tensor_copy(qmidT[:, kt, ts(tb, 128)], tp[:])
                                for kt in range(KT_KVR):
                                    tp = pt.tile([128, 128], BF, tag="pt")
                                    nc.tensor.transpose(tp[:], kvn[:, tb, ts(kt, 128)],
                                                        ident[:])
                                    nc.any.tensor_copy(kvnT[:, kt, ts(tb, 128)], tp[:])

                            # kv_a pe part: feature-major (2 heads per M-tile) + rope
                            wt_kp = w1.tile([128, KT_HID, HPC * D_ROPE], BF, tag="wkp")
                            nc.sync.dma_start(out=wt_kp[:], in_=r3(kvp_d, KT_HID))
                            for m in range(2):
                                for nck in range(2):
                                    ps = pt.tile([128, 512], F32, tag="pt")
                                    for k in range(KT_HID):
                                        nc.tensor.matmul(
                                            ps[:], lhsT=wt_kp[:, k, ts(m, 128)],
                                            rhs=xT[:, k, ts(nck, 512)],
                                            start=(k == 0), stop=(k == KT_HID - 1))
                                    rope_from_psum(ps, kpeT[:, m, ts(nck, 512)], nck, wrope)

                        # ---------- Stage 2: b-projections ----------
                        with tc.tile_pool(name="w2", bufs=1) as w2:
                            wqbn = w2.tile([128, KT_QR, HPC * D_NOPE], BF, tag="wqbn")
                            nc.sync.dma_start(out=wqbn[:], in_=r3(qbn_d, KT_QR))
                            wqbp = w2.tile([128, KT_QR, HPC * D_ROPE], BF, tag="wqbp")
                            nc.sync.dma_start(out=wqbp[:], in_=r3(qbp_d, KT_QR))
                            wkbn = w2.tile([128, KT_KVR, HPC * D_NOPE], BF, tag="wkbn")
                            nc.sync.dma_start(out=wkbn[:], in_=r3(kbn_d, KT_KVR))
                            wkbv = w2.tile([128, KT_KVR, HPC * D_V], BF, tag="wkbv")
                            nc.sync.dma_start(out=wkbv[:], in_=r3(kbv_d, KT_KVR))

                            for h in range(HPC):
                                for nck in range(2):
                                    ps = pt.tile([128, 512], F32, tag="pt")
                                    for k in range(KT_QR):
                                        nc.tensor.matmul(
                                            ps[:], lhsT=wqbn[:, k, ts(h, 128)],
                                            rhs=qmidT[:, k, ts(nck, 512)],
                                            start=(k == 0), stop=(k == KT_QR - 1))
                                    nc.any.tensor_copy(qnopeT[:, h, ts(nck, 512)], ps[:])
                            for m in range(2):
                                for nck in range(2):
                                    ps = pt.tile([128, 512], F32, tag="pt")
                                    for k in range(KT_QR):
                                        nc.tensor.matmul(
                                            ps[:], lhsT=wqbp[:, k, ts(m, 128)],
                                            rhs=qmidT[:, k, ts(nck, 512)],
                                            start=(k == 0), stop=(k == KT_QR - 1))
                                    rope_from_psum(ps, qpeT[:, m, ts(nck, 512)], nck, wrope)
                            for h in range(HPC):
                                for nck in range(2):
                                    ps = pt.tile([128, 512], F32, tag="pt")
                                    for k in range(KT_KVR):
                                        nc.tensor.matmul(
                                            ps[:], lhsT=wkbn[:, k, ts(h, 128)],
                                            rhs=kvnT[:, k, ts(nck, 512)],
                                            start=(k == 0), stop=(k == KT_KVR - 1))
                                    nc.any.tensor_copy(knopeT[:, h, ts(nck, 512)], ps[:])
                            for tb in range(TB):
                                ps = pt.tile([128, 512], F32, tag="pt")
                                for k in range(KT_KVR):
                                    nc.tensor.matmul(
                                        ps[:], lhsT=kvnT[:, k, ts(tb, 128)],
                                        rhs=wkbv[:, k, :],
                                        start=(k == 0), stop=(k == KT_KVR - 1))
                                nc.any.tensor_copy(v_s[:, tb, :], ps[:])

                # ---------- attention helper ----------
                def attention(h, tb, qnT, knT, vv, voff, ctxT, is_main, awk):
                    s0 = pp.tile([128, 512], F32, tag="pp")
                    s1 = pp.tile([128, 512], F32, tag="pp")
                    for half, sx in ((0, s0), (1, s1)):
                        nc.tensor.matmul(sx[:], lhsT=qnT[:, h, ts(tb, 128)],
                                         rhs=knT[:, h, ts(half, 512)],
                                         start=True, stop=not is_main)
                        if is_main:
                            pb = (h % 2) * 64
                            nc.tensor.matmul(
                                sx[:],
                                lhsT=qpeT[pb:pb + 64, h // 2, ts(tb, 128)],
                                rhs=kpeT[pb:pb + 64, h // 2, ts(half, 512)],
                                start=False, stop=True)
                    nm = softmax_rowstats(s0, s1)
                    probs = awk.tile([128, S], BF, tag="probs")
                    a0 = small.tile([128, 1], F32, tag="a0")
                    a1 = small.tile([128, 1], F32, tag="a1")
                    nc.scalar.activation(probs[:, 0:512], s0[:], Exp, bias=nm[:],
                                         accum_out=a0[:])
                    nc.scalar.activation(probs[:, 512:1024], s1[:], Exp, bias=nm[:],
                                         accum_out=a1[:])
                    asum = small.tile([128, 1], F32, tag="asum")
                    nc.vector.tensor_add(asum[:], a0[:], a1[:])
                    ainv = small.tile([128, 1], F32, tag="ainv")
                    nc.vector.reciprocal(ainv[:], asum[:])
                    probsT = awk.tile([128, TB, 128], BF, tag="probsT")
                    for kb in range(TB):
                        tp = pt.tile([128, 128], BF, tag="pt")
                        nc.tensor.transpose(tp[:], probs[:, ts(kb, 128)], ident[:])
                        nc.any.tensor_copy(probsT[:, kb, :], tp[:])
                    ct = pt.tile([128, 128], F32, tag="pt")
                    for kb in range(TB):
                        nc.tensor.matmul(ct[:], lhsT=probsT[:, kb, :],
                                         rhs=vv[:, kb, voff:voff + 128],
                                         start=(kb == 0), stop=(kb == TB - 1))
                    ctxn = awk.tile([128, 128], BF, tag="ctxn")
                    nc.vector.tensor_scalar_mul(ctxn[:], ct[:], ainv[:])
                    tpc = pt.tile([128, 128], BF, tag="pt")
                    nc.tensor.transpose(tpc[:], ctxn[:], ident[:])
                    nc.any.tensor_copy(ctxT[:, h, ts(tb, 128)], tpc[:])

                with tc.tile_pool(name="ctxp", bufs=1) as ctxp:
                    ctxT_m = ctxp.tile([128, HPC, S], BF, tag="ctxm")
                    ctxT_p = ctxp.tile([128, HPC, S], BF, tag="ctxp")

                    # ---------- Stage 4a: MLA attention ----------
                    with tc.tile_pool(name="awk", bufs=3) as awk:
                        for h in range(HPC):
                            for tb in range(TB):
                                attention(h, tb, qnopeT, knopeT,
                                          v_s, h * D_V, ctxT_m, True, awk)

                    # ---------- Stage 3: pattern projections + gate ----------
                    with (
                        tc.tile_pool(name="ain_pat", bufs=1) as ainp,
                        tc.tile_pool(name="w3", bufs=1) as w3,
                    ):
                        pqT = ainp.tile([128, HPC, S], BF, tag="pqT")
                        pkT = ainp.tile([128, HPC, S], BF, tag="pkT")
                        pv_s = ainp.tile([128, TB, HPC * PD], BF, tag="pv")

                        wspq = w3.tile([128, KT_HID, HPC * PD], BF, tag="w")
                        nc.sync.dma_start(out=wspq[:], in_=r3(spq_d, KT_HID))
                        for m in range(HPC):
                            for nck in range(2):
                                ps = pt.tile([128, 512], F32, tag="pt")
                                for k in range(KT_HID):
                                    nc.tensor.matmul(
                                        ps[:], lhsT=wspq[:, k, ts(m, 128)],
                                        rhs=xT[:, k, ts(nck, 512)],
                                        start=(k == 0), stop=(k == KT_HID - 1))
                                nc.any.tensor_copy(pqT[:, m, ts(nck, 512)], ps[:])
                        wspk = w3.tile([128, KT_HID, HPC * PD], BF, tag="w")
                        nc.sync.dma_start(out=wspk[:], in_=r3(spk_d, KT_HID))
                        for m in range(HPC):
                            for nck in range(2):
                                ps = pt.tile([128, 512], F32, tag="pt")
                                for k in range(KT_HID):
                                    nc.tensor.matmul(
                                        ps[:], lhsT=wspk[:, k, ts(m, 128)],
                                        rhs=xT[:, k, ts(nck, 512)],
                                        start=(k == 0), stop=(k == KT_HID - 1))
                                nc.any.tensor_copy(pkT[:, m, ts(nck, 512)], ps[:])
                        wspv = w3.tile([128, KT_HID, HPC * PD], BF, tag="w")
                        nc.sync.dma_start(out=wspv[:], in_=r3(spv_d, KT_HID))
                        for tb in range(TB):
                            ps = pt.tile([128, 512], F32, tag="pt")
                            for k in range(KT_HID):
                                nc.tensor.matmul(
                                    ps[:], lhsT=xT[:, k, ts(tb, 128)],
                                    rhs=wspv[:, k, :],
                                    start=(k == 0), stop=(k == KT_HID - 1))
                            nc.any.tensor_copy(pv_s[:, tb, :], ps[:])

                        # gate
                        gwt = w3.tile([128, KT_HID, 2], BF, tag="gw")
                        nc.sync.dma_start(out=gwt[:], in_=r3(gw_d, KT_HID))
                        for tb in range(TB):
                            psg = pp.tile([128, 2], F32, tag="pp")
                            for k in range(KT_HID):
                                nc.tensor.matmul(psg[:], lhsT=xT[:, k, ts(tb, 128)],
                                                 rhs=gwt[:, k, :],
                                                 start=(k == 0), stop=(k == KT_HID - 1))
                            glog = small.tile([128, 2], F32, tag="glog")
                            nc.vector.tensor_add(glog[:], psg[:], gbias[:])
                            gm = small.tile([128, 1], F32, tag="gm")
                            nc.vector.reduce_max(gm[:], glog[:], axis=X)
                            nc.vector.tensor_scalar_mul(gm[:], gm[:], -1.0)
                            gexp = small.tile([128, 2], F32, tag="gexp")
                            gsum = small.tile([128, 1], F32, tag="gsum")
                            nc.scalar.activation(gexp[:], glog[:], Exp, bias=gm[:],
                                                 accum_out=gsum[:])
                            ginv = small.tile([128, 1], F32, tag="ginv")
                            nc.vector.reciprocal(ginv[:], gsum[:])
                            nc.vector.tensor_scalar_mul(g0_s[:, tb:tb + 1],
                                                        gexp[:, 0:1], ginv[:])
                            nc.vector.tensor_scalar_mul(g1_s[:, tb:tb + 1],
                                                        gexp[:, 1:2], ginv[:])

                        # ---------- Stage 4b: pattern attention ----------
                        with tc.tile_pool(name="awk2", bufs=3) as awk2:
                            for h in range(HPC):
                                for tb in range(TB):
                                    attention(h, tb, pqT, pkT,
                                              pv_s, h * PD, ctxT_p, False, awk2)

                    # ---------- Stage 5: output projections + gate combine ----------
                    with (
                        tc.tile_pool(name="w5", bufs=1) as w5,
                        tc.tile_pool(name="ow", bufs=2) as ow,
                    ):
                        wo = w5.tile([128, KT_KVR, HID], BF, tag="wo")
                        nc.sync.dma_start(out=wo[:], in_=r3(ow_d, KT_KVR))
                        wspo = w5.tile([128, KT_KVR, HID], BF, tag="wspo")
                        nc.sync.dma_start(out=wspo[:], in_=r3(spo_d, KT_KVR))
                        for tb in range(TB):
                            osb = ow.tile([128, HID], F32, tag="osb")
                            for ck in range(4):
                                pm = pp.tile([128, 512], F32, tag="pp")
                                for k in range(KT_KVR):
                                    nc.tensor.matmul(
                                        pm[:], lhsT=ctxT_m[:, k, ts(tb, 128)],
                                        rhs=wo[:, k, ts(ck, 512)],
                                        start=(k == 0), stop=(k == KT_KVR - 1))
                                pq2 = pp.tile([128, 512], F32, tag="pp")
                                for k in range(KT_KVR):
                                    nc.tensor.matmul(
                                        pq2[:], lhsT=ctxT_p[:, k, ts(tb, 128)],
                                        rhs=wspo[:, k, ts(ck, 512)],
                                        start=(k == 0), stop=(k == KT_KVR - 1))
                                tmp = ow.tile([128, 512], F32, tag="tmp")
                                nc.vector.tensor_scalar_mul(tmp[:], pq2[:],
                                                            g1_s[:, tb:tb + 1])
                                nc.vector.scalar_tensor_tensor(
                                    osb[:, ts(ck, 512)], in0=pm[:],
                                    scalar=g0_s[:, tb:tb + 1],
                                    in1=tmp[:], op0=MULT, op1=ADD)
                            nc.sync.dma_start(out=out_d[ts(tb, 128), :], in_=osb[:])

    nc.compile()
    return nc


def _rope_tables():
    inv_freq = 1.0 / (THETA ** (np.arange(0, D_ROPE, 2, dtype=np.float32) / D_ROPE))
    t = np.arange(S, dtype=np.float32)
    freqs = np.outer(t, inv_freq)                       # [S, 32]
    emb = np.concatenate([freqs, freqs], -1)            # [S, 64]
    cosT = np.cos(emb).T.astype(np.float32)             # [64, S]
    sinT = np.sin(emb).T.astype(np.float32)
    cos2T = np.ascontiguousarray(np.concatenate([cosT, cosT], 0))   # [128, S]
    sin2T = np.ascontiguousarray(np.concatenate([sinT, sinT], 0))
    return cos2T.astype(BF16), sin2T.astype(BF16)


def _prep_in_maps(hidden_states, q_a_w, q_a_ln_w, q_b_w, kv_a_w, kv_a_ln_w,
                  kv_b_w, o_w, sp_q_w, sp_k_w, sp_v_w, sp_o_w, gate_w, gate_b):
    def bf(x):
        return np.ascontiguousarray(x).astype(BF16)

    cos2T, sin2T = _rope_tables()
    qa_wT = bf(q_a_w.T)                                   # [HID, QR]
    kvl_wT = bf(kv_a_w[:KVR].T)                           # [HID, KVR]
    kv_a_pe = kv_a_w[KVR:].reshape(H, D_ROPE, HID)        # [H, 64, HID]

    qb = (q_b_w * q_a_ln_w[None, :]).reshape(H, D_Q, QR) * (D_Q ** -0.5)
    qb_nope = qb[:, :D_NOPE]                              # [H,128,QR]
    qb_pe = qb[:, D_NOPE:]                                # [H,64,QR]
    kvb = (kv_b_w * kv_a_ln_w[None, :]).reshape(H, D_NOPE + D_V, KVR)
    kb_nope = kvb[:, :D_NOPE]                             # [H,128,KVR]
    kb_v = kvb[:, D_NOPE:]                                # [H,128,KVR]
    o_wh = o_w.reshape(HID, H, D_V)                       # [HID,H,128]
    spq = (sp_q_w * (PD ** -0.5)).reshape(PH, PD, HID)
    spk = sp_k_w.reshape(PH, PD, HID)
    spv = sp_v_w.reshape(PH, PD, HID)
    spo = sp_o_w.reshape(HID, PH, PD)
    gate_wT = bf(gate_w.T)                                # [HID, 2]
    gate_bias = np.ascontiguousarray(
        np.broadcast_to(gate_b[None, :], (128, 2))).astype(np.float32)

    in_maps = []
    for c in range(NCORES):
        b, g = c // 4, c % 4
        hs = slice(4 * g, 4 * g + 4)
        m = {
            "xT": bf(hidden_states[b].T),
            "qa_wT": qa_wT,
            "qbn_wT": bf(qb_nope[hs].reshape(HPC * D_NOPE, QR).T),
            "qbp_wT": bf(qb_pe[hs].reshape(HPC * D_ROPE, QR).T),
            "kvl_wT": kvl_wT,
            "kvp_wT": bf(kv_a_pe[hs].reshape(HPC * D_ROPE, HID).T),
            "kbn_wT": bf(kb_nope[hs].reshape(HPC * D_NOPE, KVR).T),
            "kbv_wT": bf(kb_v[hs].reshape(HPC * D_V, KVR).T),
            "o_wT": bf(o_wh[:, hs].reshape(HID, HPC * D_V).T),
            "spq_wT": bf(spq[hs].reshape(HPC * PD, HID).T),
            "spk_wT": bf(spk[hs].reshape(HPC * PD, HID).T),
            "spv_wT": bf(spv[hs].reshape(HPC * PD, HID).T),
            "spo_wT": bf(spo[:, hs].reshape(HID, HPC * PD).T),
            "gate_wT": gate_wT,
            "gate_bias": gate_bias,
            "cos2T": cos2T,
            "sin2T": sin2T,
        }
        in_maps.append(m)
    return in_maps


def kernel(**inputs):
    global LAST_RESULT
    from concourse.bass_utils import run_bass_kernel_spmd

    inputs = {k: np.asarray(v) for k, v in inputs.items()}
    if "nc" not in _graph_cache:
        _graph_cache["nc"] = _build_graph()
    nc = _graph_cache["nc"]

    in_maps = _prep_in_maps(**inputs)
    res = run_bass_kernel_spmd(nc, in_maps, core_ids=list(range(NCORES)),
                               trace=TRACE, **RUN_KWARGS)
    LAST_RESULT = res
    out = np.zeros((B, S, HID), np.float32)
    for c in range(NCORES):
        out[c // 4] += res.results[c]["out"]
    return out


# revision 14
# speedup vs baseline: 1.0307x; 1.0307x over previous
"""ARCAttention (MLA + pattern-attention + gate) distributed Bass kernel for 8 TRN2 NeuronCores.

Sharding: data-parallel over batch (B=2) x tensor-parallel over heads (4 head-groups).
Core c handles batch (c // 4), heads [4*(c%4) .. 4*(c%4)+4) of both the MLA path and the
pattern path. The low-rank a-projections (q_a, kv_a lora) and the gate are replicated
within a batch group. Each core emits a partial (already gate-weighted) output
[S, HID]; the host sums the 4 partials per batch. No device collectives.

All matmuls run in bf16 (f32 PSUM accumulation); softmax/rmsnorm statistics in f32.
Weight preprocessing (transposes, ln-weight folding, scale folding, rope tables) is
done on host in numpy and shipped per-core via in_maps.
"""

import numpy as np
import ml_dtypes

# ---- model config (hardcoded from the problem spec) ----
B, S, HID = 2, 1024, 2048
H = 16
D_NOPE, D_ROPE, D_V = 128, 64, 128
D_Q = D_NOPE + D_ROPE            # 192
QR, KVR = 1536, 512
PH, PD = 16, 128
THETA, EPS = 10000.0, 1e-6
NCORES = 8
HPC = 4                          # heads per core
TB = S // 128                    # 8 token blocks
KT_HID = HID // 128              # 16
KT_QR = QR // 128                # 12
KT_KVR = KVR // 128              # 4

BF16 = ml_dtypes.bfloat16

# knobs for test harness
TRACE = False
RUN_KWARGS = {}
LAST_RESULT = None

_graph_cache = {}


def _build_graph():
    import concourse.bass as bass
    import concourse.mybir as mybir
    import concourse.tile as tile
    from concourse import bacc
    from concourse.masks import make_identity

    BF = mybir.dt.bfloat16
    F32 = mybir.dt.float32
    Exp = mybir.ActivationFunctionType.Exp
    Square = mybir.ActivationFunctionType.Square
    Sqrt = mybir.ActivationFunctionType.Sqrt
    MULT = mybir.AluOpType.mult
    ADD = mybir.AluOpType.add
    X = mybir.AxisListType.X
    ts = bass.ts

    nc = bacc.Bacc("TRN2", target_bir_lowering=False, debug=False,
                   num_devices=NCORES)

    def din(name, shape, dt=BF):
        return nc.declare_dram_parameter(name, list(shape), dt, isOutput=False)

    xT_d = din("xT", [HID, S])
    qa_d = din("qa_wT", [HID, QR])
    qbn_d = din("qbn_wT", [QR, HPC * D_NOPE])
    qbp_d = din("qbp_wT", [QR, HPC * D_ROPE])
    kvl_d = din("kvl_wT", [HID, KVR])
    kvp_d = din("kvp_wT", [HID, HPC * D_ROPE])
    kbn_d = din("kbn_wT", [KVR, HPC * D_NOPE])
    kbv_d = din("kbv_wT", [KVR, HPC * D_V])
    ow_d = din("o_wT", [HPC * D_V, HID])
    spq_d = din("spq_wT", [HID, HPC * PD])
    spk_d = din("spk_wT", [HID, HPC * PD])
    spv_d = din("spv_wT", [HID, HPC * PD])
    spo_d = din("spo_wT", [HPC * PD, HID])
    gw_d = din("gate_wT", [HID, 2])
    gb_d = din("gate_bias", [128, 2], F32)
    cos_d = din("cos2T", [128, S])
    sin_d = din("sin2T", [128, S])
    out_d = nc.declare_dram_parameter("out", [S, HID], F32, isOutput=True)

    def r3(dram, kt):
        # [kt*128, N] dram tensor viewed as [128, kt, N] for SBUF tiling
        return dram.ap().rearrange("(k p) n -> p k n", p=128, k=kt)

    with tile.TileContext(nc) as tc:
        with (
            tc.tile_pool(name="const", bufs=1) as constp,
            tc.tile_pool(name="small", bufs=4) as small,
            tc.tile_pool(name="pp", bufs=4, space="PSUM") as pp,
            tc.tile_pool(name="pt", bufs=3, space="PSUM") as pt,
            tc.tile_pool(name="psum1", bufs=1, space="PSUM") as psum1,
            tc.tile_pool(name="ain_mla", bufs=1) as ain,
        ):
            ident = constp.tile([128, 128], BF, tag="ident")
            make_identity(nc, ident[:])
            eps_t = constp.tile([128, 1], F32, tag="eps")
            nc.vector.memset(eps_t[:], EPS)
            ones_col = constp.tile([128, 1], BF, tag="ones_col")
            nc.vector.memset(ones_col[:], 1.0)
            ones_row = constp.tile([1, 128], BF, tag="ones_row")
            nc.vector.memset(ones_row[:], 1.0)
            cosT = constp.tile([128, S], BF, tag="cos")
            sinT = constp.tile([128, S], BF, tag="sin")
            nc.sync.dma_start(out=cosT[:], in_=cos_d.ap())
            nc.sync.dma_start(out=sinT[:], in_=sin_d.ap())
            gbias = constp.tile([128, 2], F32, tag="gb")
            nc.sync.dma_start(out=gbias[:], in_=gb_d.ap())
            g0_s = constp.tile([128, TB], F32, tag="g0")
            g1_s = constp.tile([128, TB], F32, tag="g1")
            ssq_q = constp.tile([128, TB * 3], F32, tag="ssq_q")
            ssq_k = constp.tile([128, TB], F32, tag="ssq_k")

            # MLA attention inputs (feature-major unless noted)
            qnopeT = ain.tile([128, HPC, S], BF, tag="qnopeT")
            qpeT = ain.tile([128, 2, S], BF, tag="qpeT")
            knopeT = ain.tile([128, HPC, S], BF, tag="knopeT")
            kpeT = ain.tile([128, 2, S], BF, tag="kpeT")
            v_s = ain.tile([128, TB, HPC * D_V], BF, tag="v")      # token-major

            def rope_from_psum(ps, dst, nck, work):
                """Apply rope to a [128, 512] psum chunk holding 2 stacked
                64-dim pe heads; write bf16 to dst ([128,512] slice)."""
                rot = work.tile([128, 512], F32, tag="rot")
                nc.vector.tensor_scalar_mul(rot[0:32, :], ps[32:64, :], -1.0)
                nc.vector.tensor_copy(rot[32:64, :], ps[0:32, :])
                nc.vector.tensor_scalar_mul(rot[64:96, :], ps[96:128, :], -1.0)
                nc.vector.tensor_copy(rot[96:128, :], ps[64:96, :])
                t1 = work.tile([128, 512], F32, tag="t1")
                nc.vector.tensor_mul(t1[:], ps[:], cosT[:, ts(nck, 512)])
                nc.vector.tensor_mul(rot[:], rot[:], sinT[:, ts(nck, 512)])
                nc.vector.tensor_add(dst, t1[:], rot[:])

            with tc.tile_pool(name="xp", bufs=1) as xp:
                xT = xp.tile([128, KT_HID, S], BF, tag="xT")
                nc.sync.dma_start(out=xT[:], in_=r3(xT_d, KT_HID))

                with tc.tile_pool(name="wrope", bufs=2) as wrope:
                    # ---------- Stage 1+2 ----------
                    with tc.tile_pool(name="q2", bufs=1) as q2:
                        qmidT = q2.tile([128, KT_QR, S], BF, tag="qmidT")
                        kvnT = q2.tile([128, KT_KVR, S], BF, tag="kvnT")

                        with (
                            tc.tile_pool(name="q1", bufs=1) as q1,
                            tc.tile_pool(name="w1", bufs=1) as w1,
                        ):
                            qmid = q1.tile([128, TB, QR], BF, tag="qmid")
                            kvn = q1.tile([128, TB, KVR], BF, tag="kvn")

                            # q_a: token-major [tok, QR] in 3 chunks of 512
                            for ck in range(3):
                                wt = w1.tile([128, KT_HID, 512], BF, tag="w")
                                nc.sync.dma_start(
                                    out=wt[:], in_=r3(qa_d, KT_HID)[:, :, ts(ck, 512)])
                                for tb in range(TB):
                                    ps = pp.tile([128, 512], F32, tag="pp")
                                    for k in range(KT_HID):
                                        nc.tensor.matmul(
                                            ps[:], lhsT=xT[:, k, ts(tb, 128)],
                                            rhs=wt[:, k, :],
                                            start=(k == 0), stop=(k == KT_HID - 1))
                                    nc.any.tensor_copy(qmid[:, tb, ts(ck, 512)], ps[:])
                                    nc.scalar.activation(
                                        ps[:], ps[:], Square,
                                        accum_out=ssq_q[:, tb * 3 + ck: tb * 3 + ck + 1])

                            # kv_a lora part: token-major [tok, KVR]
                            wt_kl = w1.tile([128, KT_HID, 512], BF, tag="w")
                            nc.sync.dma_start(out=wt_kl[:], in_=r3(kvl_d, KT_HID))
                            for tb in range(TB):
                                ps = pp.tile([128, 512], F32, tag="pp")
                                for k in range(KT_HID):
                                    nc.tensor.matmul(
                                        ps[:], lhsT=xT[:, k, ts(tb, 128)],
                                        rhs=wt_kl[:, k, :],
                                        start=(k == 0), stop=(k == KT_HID - 1))
                                nc.any.tensor_copy(kvn[:, tb, :], ps[:])
                                nc.scalar.activation(ps[:], ps[:], Square,
                                                     accum_out=ssq_k[:, tb:tb + 1])

                            # rmsnorm scales (in-place), then transpose to feature-major
                            for tb in range(TB):
                                acc = small.tile([128, 1], F32, tag="acc")
                                nc.vector.tensor_add(acc[:], ssq_q[:, tb * 3:tb * 3 + 1],
                                                     ssq_q[:, tb * 3 + 1:tb * 3 + 2])
                                nc.vector.tensor_add(acc[:], acc[:],
                                                     ssq_q[:, tb * 3 + 2:tb * 3 + 3])
                                rms = small.tile([128, 1], F32, tag="rms")
                                nc.scalar.activation(rms[:], acc[:], Sqrt,
                                                     bias=eps_t[:], scale=1.0 / QR)
                                inv = small.tile([128, 1], F32, tag="inv")
                                nc.vector.reciprocal(inv[:], rms[:])
                                nc.vector.tensor_scalar_mul(qmid[:, tb, :],
                                                            qmid[:, tb, :], inv[:])

                                rms2 = small.tile([128, 1], F32, tag="rms")
                                nc.scalar.activation(rms2[:], ssq_k[:, tb:tb + 1], Sqrt,
                                                     bias=eps_t[:], scale=1.0 / KVR)
                                inv2 = small.tile([128, 1], F32, tag="inv")
                                nc.vector.reciprocal(inv2[:], rms2[:])
                                nc.vector.tensor_scalar_mul(kvn[:, tb, :],
                                                            kvn[:, tb, :], inv2[:])

                                for kt in range(KT_QR):
                                    tp = pt.tile([128, 128], BF, tag="pt")
                                    nc.tensor.transpose(tp[:], qmid[:, tb, ts(kt, 128)],
                                                        ident[:])
                                    nc.any.tensor_copy(qmidT[:, kt, ts(tb, 128)], tp[:])
                                for kt in range(KT_KVR):
                                    tp = pt.tile([128, 128], BF, tag="pt")
                                    nc.tensor.transpose(tp[:], kvn[:, tb, ts(kt, 128)],
                                                        ident[:])
                                    nc.any.tensor_copy(kvnT[:, kt, ts(tb, 128)], tp[:])

                            # kv_a pe part: feature-major (2 heads per M-tile) + rope
                            wt_kp = w1.tile([128, KT_HID, HPC * D_ROPE], BF, tag="wkp")
                            nc.sync.dma_start(out=wt_kp[:], in_=r3(kvp_d, KT_HID))
                            for m in range(2):
                                for nck in range(2):
                                    ps = pt.tile([128, 512], F32, tag="pt")
                                    for k in range(KT_HID):
                                        nc.tensor.matmul(
                                            ps[:], lhsT=wt_kp[:, k, ts(m, 128)],
                                            rhs=xT[:, k, ts(nck, 512)],
                                            start=(k == 0), stop=(k == KT_HID - 1))
                                    rope_from_psum(ps, kpeT[:, m, ts(nck, 512)], nck, wrope)

                        # ---------- Stage 2: b-projections ----------
                        with tc.tile_pool(name="w2", bufs=1) as w2:
                            wqbn = w2.tile([128, KT_QR, HPC * D_NOPE], BF, tag="wqbn")
                            nc.sync.dma_start(out=wqbn[:], in_=r3(qbn_d, KT_QR))
                            wqbp = w2.tile([128, KT_QR, HPC * D_ROPE], BF, tag="wqbp")
                            nc.sync.dma_start(out=wqbp[:], in_=r3(qbp_d, KT_QR))
                            wkbn = w2.tile([128, KT_KVR, HPC * D_NOPE], BF, tag="wkbn")
                            nc.sync.dma_start(out=wkbn[:], in_=r3(kbn_d, KT_KVR))
                            wkbv = w2.tile([128, KT_KVR, HPC * D_V], BF, tag="wkbv")
                            nc.sync.dma_start(out=wkbv[:], in_=r3(kbv_d, KT_KVR))

                            for h in range(HPC):
                                for nck in range(2):
                                    ps = pt.tile([128, 512], F32, tag="pt")
                                    for k in range(KT_QR):
                                        nc.tensor.matmul(
                                            ps[:], lhsT=wqbn[:, k, ts(h, 128)],
                                            rhs=qmidT[:, k, ts(nck, 512)],
                                            start=(k == 0), stop=(k == KT_QR - 1))
                                    nc.any.tensor_copy(qnopeT[:, h, ts(nck, 512)], ps[:])
                            for m in range(2):
                                for nck in range(2):
                                    ps = pt.tile([128, 512], F32, tag="pt")
                                    for k in range(KT_QR):
                                        nc.tensor.matmul(
                                            ps[:], lhsT=wqbp[:, k, ts(m, 128)],
                                            rhs=qmidT[:, k, ts(nck, 512)],
                                            start=(k == 0), stop=(k == KT_QR - 1))
                                    rope_from_psum(ps, qpeT[:, m, ts(nck, 512)], nck, wrope)
                            for h in range(HPC):
                                for nck in range(2):
                                    ps = pt.tile([128, 512], F32, tag="pt")
                                    for k in range(KT_KVR):
                                        nc.tensor.matmul(
                                            ps[:], lhsT=wkbn[:, k, ts(h, 128)],
                                            rhs=kvnT[:, k, ts(nck, 512)],
                                            start=(k == 0), stop=(k == KT_KVR - 1))
                                    nc.any.tensor_copy(knopeT[:, h, ts(nck, 512)], ps[:])
                            for tb in range(TB):
                                ps = pt.tile([128, 512], F32, tag="pt")
                                for k in range(KT_KVR):
                                    nc.tensor.matmul(
                                        ps[:], lhsT=kvnT[:, k, ts(tb, 128)],
                                        rhs=wkbv[:, k, :],
                                        start=(k == 0), stop=(k == KT_KVR - 1))
                                nc.any.tensor_copy(v_s[:, tb, :], ps[:])

                # ---------- attention helper ----------
                # k-major formulation: scoresT[k,q] on PE, unnormalized exp
                # (|score| <= ||q||*||k||/sqrt(D) stays well inside f32 exp
                # range for this model), v-stationary ctx matmuls at N=512,
                # softmax denominators via PE ones-reduction, normalization
                # via an outer-product broadcast.
                def attention(h, qh, qnT, knT, vv, voff, ctxT, is_main, awk):
                    probsT = awk.tile([128, TB, 512], BF, tag="probsT")
                    for kb in range(TB):
                        ps = pp.tile([128, 512], F32, tag="pp")
                        nc.tensor.matmul(ps[:], lhsT=knT[:, h, ts(kb, 128)],
                                         rhs=qnT[:, h, ts(qh, 512)],
                                         start=True, stop=not is_main)
                        if is_main:
                            pb = (h % 2) * 64
                            nc.tensor.matmul(
                                ps[:],
                                lhsT=kpeT[pb:pb + 64, h // 2, ts(kb, 128)],
                                rhs=qpeT[pb:pb + 64, h // 2, ts(qh, 512)],
                                start=False, stop=True)
                        nc.scalar.activation(probsT[:, kb, :], ps[:], Exp)
                    ct = pt.tile([128, 512], F32, tag="pt")
                    for kb in range(TB):
                        nc.tensor.matmul(ct[:], lhsT=vv[:, kb, voff:voff + 128],
                                         rhs=probsT[:, kb, :],
                                         start=(kb == 0), stop=(kb == TB - 1))
                    sm = psum1.tile([1, 512], F32, tag="sm")
                    for kb in range(TB):
                        nc.tensor.matmul(sm[:], lhsT=ones_col[:],
                                         rhs=probsT[:, kb, :],
                                         start=(kb == 0), stop=(kb == TB - 1))
                    inv = small.tile([1, 512], F32, tag="invrow")
                    nc.vector.reciprocal(inv[:], sm[:])
                    invb = small.tile([1, 512], BF, tag="invrowb")
                    nc.vector.tensor_copy(invb[:], inv[:])
                    bc = pt.tile([128, 512], F32, tag="pt")
                    nc.tensor.matmul(bc[:], lhsT=ones_row[:], rhs=invb[:],
                                     start=True, stop=True)
                    bcs = awk.tile([128, 512], F32, tag="bcs")
                    nc.any.tensor_copy(bcs[:], bc[:])
                    nc.vector.tensor_mul(ctxT[:, h, ts(qh, 512)], ct[:], bcs[:])

                with tc.tile_pool(name="ctxp", bufs=1) as ctxp:
                    ctxT_m = ctxp.tile([128, HPC, S], BF, tag="ctxm")
                    ctxT_p = ctxp.tile([128, HPC, S], BF, tag="ctxp")

                    # ---------- Stage 4a: MLA attention ----------
                    with tc.tile_pool(name="awk", bufs=2) as awk:
                        for h in range(HPC):
                            for qh in range(2):
                                attention(h, qh, qnopeT, knopeT,
                                          v_s, h * D_V, ctxT_m, True, awk)

                    # ---------- Stage 3: pattern projections + gate ----------
                    with (
                        tc.tile_pool(name="ain_pat", bufs=1) as ainp,
                        tc.tile_pool(name="w3", bufs=1) as w3,
                    ):
                        pqT = ainp.tile([128, HPC, S], BF, tag="pqT")
                        pkT = ainp.tile([128, HPC, S], BF, tag="pkT")
                        pv_s = ainp.tile([128, TB, HPC * PD], BF, tag="pv")

                        wspq = w3.tile([128, KT_HID, HPC * PD], BF, tag="w")
                        nc.sync.dma_start(out=wspq[:], in_=r3(spq_d, KT_HID))
                        for m in range(HPC):
                            for nck in range(2):
                                ps = pt.tile([128, 512], F32, tag="pt")
                                for k in range(KT_HID):
                                    nc.tensor.matmul(
                                        ps[:], lhsT=wspq[:, k, ts(m, 128)],
                                        rhs=xT[:, k, ts(nck, 512)],
                                        start=(k == 0), stop=(k == KT_HID - 1))
                                nc.any.tensor_copy(pqT[:, m, ts(nck, 512)], ps[:])
                        wspk = w3.tile([128, KT_HID, HPC * PD], BF, tag="w")
                        nc.sync.dma_start(out=wspk[:], in_=r3(spk_d, KT_HID))
                        for m in range(HPC):
                            for nck in range(2):
                                ps = pt.tile([128, 512], F32, tag="pt")
                                for k in range(KT_HID):
                                    nc.tensor.matmul(
                                        ps[:], lhsT=wspk[:, k, ts(m, 128)],
                                        rhs=xT[:, k, ts(nck, 512)],
                                        start=(k == 0), stop=(k == KT_HID - 1))
                                nc.any.tensor_copy(pkT[:, m, ts(nck, 512)], ps[:])
                        wspv = w3.tile([128, KT_HID, HPC * PD], BF, tag="w")
                        nc.sync.dma_start(out=wspv[:], in_=r3(spv_d, KT_HID))
                        for tb in range(TB):
                            ps = pt.tile([128, 512], F32, tag="pt")
                            for k in range(KT_HID):
                                nc.tensor.matmul(
                                    ps[:], lhsT=xT[:, k, ts(tb, 128)],
                                    rhs=wspv[:, k, :],
                                    start=(k == 0), stop=(k == KT_HID - 1))
                            nc.any.tensor_copy(pv_s[:, tb, :], ps[:])

                        # gate
                        gwt = w3.tile([128, KT_HID, 2], BF, tag="gw")
                        nc.sync.dma_start(out=gwt[:], in_=r3(gw_d, KT_HID))
                        for tb in range(TB):
                            psg = pp.tile([128, 2], F32, tag="pp")
                            for k in range(KT_HID):
                                nc.tensor.matmul(psg[:], lhsT=xT[:, k, ts(tb, 128)],
                                                 rhs=gwt[:, k, :],
                                                 start=(k == 0), stop=(k == KT_HID - 1))
                            glog = small.tile([128, 2], F32, tag="glog")
                            nc.vector.tensor_add(glog[:], psg[:], gbias[:])
                            gm = small.tile([128, 1], F32, tag="gm")
                            nc.vector.reduce_max(gm[:], glog[:], axis=X)
                            nc.vector.tensor_scalar_mul(gm[:], gm[:], -1.0)
                            gexp = small.tile([128, 2], F32, tag="gexp")
                            gsum = small.tile([128, 1], F32, tag="gsum")
                            nc.scalar.activation(gexp[:], glog[:], Exp, bias=gm[:],
                                                 accum_out=gsum[:])
                            ginv = small.tile([128, 1], F32, tag="ginv")
                            nc.vector.reciprocal(ginv[:], gsum[:])
                            nc.vector.tensor_scalar_mul(g0_s[:, tb:tb + 1],
                                                        gexp[:, 0:1], ginv[:])
                            nc.vector.tensor_scalar_mul(g1_s[:, tb:tb + 1],
                                                        gexp[:, 1:2], ginv[:])

                        # ---------- Stage 4b: pattern attention ----------
                        with tc.tile_pool(name="awk2", bufs=2) as awk2:
                            for h in range(HPC):
                                for qh in range(2):
                                    attention(h, qh, pqT, pkT,
                                              pv_s, h * PD, ctxT_p, False, awk2)

                    # ---------- Stage 5: output projections + gate combine ----------
                    with (
                        tc.tile_pool(name="w5", bufs=1) as w5,
                        tc.tile_pool(name="ow", bufs=2) as ow,
                    ):
                        wo = w5.tile([128, KT_KVR, HID], BF, tag="wo")
                        nc.sync.dma_start(out=wo[:], in_=r3(ow_d, KT_KVR))
                        wspo = w5.tile([128, KT_KVR, HID], BF, tag="wspo")
                        nc.sync.dma_start(out=wspo[:], in_=r3(spo_d, KT_KVR))
                        for tb in range(TB):
                            osb = ow.tile([128, HID], F32, tag="osb")
                            for ck in range(4):
                                pm = pp.tile([128, 512], F32, tag="pp")
                                for k in range(KT_KVR):
                                    nc.tensor.matmul(
                                        pm[:], lhsT=ctxT_m[:, k, ts(tb, 128)],
                                        rhs=wo[:, k, ts(ck, 512)],
                                        start=(k == 0), stop=(k == KT_KVR - 1))
                                pq2 = pp.tile([128, 512], F32, tag="pp")
                                for k in range(KT_KVR):
                                    nc.tensor.matmul(
                                        pq2[:], lhsT=ctxT_p[:, k, ts(tb, 128)],
                                        rhs=wspo[:, k, ts(ck, 512)],
                                        start=(k == 0), stop=(k == KT_KVR - 1))
                                tmp = ow.tile([128, 512], F32, tag="tmp")
                                nc.vector.tensor_scalar_mul(tmp[:], pq2[:],
                                                            g1_s[:, tb:tb + 1])
                                nc.vector.scalar_tensor_tensor(
                                    osb[:, ts(ck, 512)], in0=pm[:],
                                    scalar=g0_s[:, tb:tb + 1],
                                    in1=tmp[:], op0=MULT, op1=ADD)
                            nc.sync.dma_start(out=out_d[ts(tb, 128), :], in_=osb[:])

    nc.compile()
    return nc


def _rope_tables():
    inv_freq = 1.0 / (THETA ** (np.arange(0, D_ROPE, 2, dtype=np.float32) / D_ROPE))
    t = np.arange(S, dtype=np.float32)
    freqs = np.outer(t, inv_freq)                       # [S, 32]
    emb = np.concatenate([freqs, freqs], -1)            # [S, 64]
    cosT = np.cos(emb).T.astype(np.float32)             # [64, S]
    sinT = np.sin(emb).T.astype(np.float32)
    cos2T = np.ascontiguousarray(np.concatenate([cosT, cosT], 0))   # [128, S]
    sin2T = np.ascontiguousarray(np.concatenate([sinT, sinT], 0))
    return cos2T.astype(BF16), sin2T.astype(BF16)


def _prep_in_maps(hidden_states, q_a_w, q_a_ln_w, q_b_w, kv_a_w, kv_a_ln_w,
                  kv_b_w, o_w, sp_q_w, sp_k_w, sp_v_w, sp_o_w, gate_w, gate_b):
    def bf(x):
        return np.ascontiguousarray(x).astype(BF16)

    cos2T, sin2T = _rope_tables()
    qa_wT = bf(q_a_w.T)                                   # [HID, QR]
    kvl_wT = bf(kv_a_w[:KVR].T)                           # [HID, KVR]
    kv_a_pe = kv_a_w[KVR:].reshape(H, D_ROPE, HID)        # [H, 64, HID]

    qb = (q_b_w * q_a_ln_w[None, :]).reshape(H, D_Q, QR) * (D_Q ** -0.5)
    qb_nope = qb[:, :D_NOPE]                              # [H,128,QR]
    qb_pe = qb[:, D_NOPE:]                                # [H,64,QR]
    kvb = (kv_b_w * kv_a_ln_w[None, :]).reshape(H, D_NOPE + D_V, KVR)
    kb_nope = kvb[:, :D_NOPE]                             # [H,128,KVR]
    kb_v = kvb[:, D_NOPE:]                                # [H,128,KVR]
    o_wh = o_w.reshape(HID, H, D_V)                       # [HID,H,128]
    spq = (sp_q_w * (PD ** -0.5)).reshape(PH, PD, HID)
    spk = sp_k_w.reshape(PH, PD, HID)
    spv = sp_v_w.reshape(PH, PD, HID)
    spo = sp_o_w.reshape(HID, PH, PD)
    gate_wT = bf(gate_w.T)                                # [HID, 2]
    gate_bias = np.ascontiguousarray(
        np.broadcast_to(gate_b[None, :], (128, 2))).astype(np.float32)

    in_maps = []
    for c in range(NCORES):
        b, g = c // 4, c % 4
        hs = slice(4 * g, 4 * g + 4)
        m = {
            "xT": bf(hidden_states[b].T),
            "qa_wT": qa_wT,
            "qbn_wT": bf(qb_nope[hs].reshape(HPC * D_NOPE, QR).T),
            "qbp_wT": bf(qb_pe[hs].reshape(HPC * D_ROPE, QR).T),
            "kvl_wT": kvl_wT,
            "kvp_wT": bf(kv_a_pe[hs].reshape(HPC * D_ROPE, HID).T),
            "kbn_wT": bf(kb_nope[hs].reshape(HPC * D_NOPE, KVR).T),
            "kbv_wT": bf(kb_v[hs].reshape(HPC * D_V, KVR).T),
            "o_wT": bf(o_wh[:, hs].reshape(HID, HPC * D_V).T),
            "spq_wT": bf(spq[hs].reshape(HPC * PD, HID).T),
            "spk_wT": bf(spk[hs].reshape(HPC * PD, HID).T),
            "spv_wT": bf(spv[hs].reshape(HPC * PD, HID).T),
            "spo_wT": bf(spo[:, hs].reshape(HID, HPC * PD).T),
            "gate_wT": gate_wT,
            "gate_bias": gate_bias,
            "cos2T": cos2T,
            "sin2T": sin2T,
        }
        in_maps.append(m)
    return in_maps


def kernel(**inputs):
    global LAST_RESULT
    from concourse.bass_utils import run_bass_kernel_spmd

    inputs = {k: np.asarray(v) for k, v in inputs.items()}
    if "nc" not in _graph_cache:
        _graph_cache["nc"] = _build_graph()
    nc = _graph_cache["nc"]

    in_maps = _prep_in_maps(**inputs)
    res = run_bass_kernel_spmd(nc, in_maps, core_ids=list(range(NCORES)),
                               trace=TRACE, **RUN_KWARGS)
    LAST_RESULT = res
    out = np.zeros((B, S, HID), np.float32)
    for c in range(NCORES):
        out[c // 4] += res.results[c]["out"]
    return out
